# revision 19
# baseline (speedup 1.0000x reference)
"""KNN graph kernel for Trainium2 (8 NeuronCores).

Problem: x [16384, 128] f32 -> indices of the 16 nearest neighbors per row
(excluding self), int32 [16384, 16], matching
    d2 = |x_i|^2 + |x_j|^2 - 2 x_i.x_j ;  top_k(-sqrt(max(d2,0)), 17)[:, 1:17]

Approach (per core, rows sharded 2048/core):
  score p[m, n] = 2 x_m.x_n - |x_n|^2  (row-monotone transform of -d2, so
  per-row ordering matches; self is the strict row max by ~|x_m|^2 margin).
  PE: fp32 matmuls produce p in PSUM [128, 512] chunks (K=128 main matmul
  plus a K=1 accumulate folding in -|x_n|^2).
  DVE: per chunk, Max (top-8 values) + MaxIndex (their positions); then a
  3-round Max/MaxIndex/MatchReplace merge over the 32*8=256 candidates
  gives the top-17 (slot 0 = self).
  Host: maps candidate positions back to global column indices, drops the
  self slot, and exactly recomputes the rare rows flagged as ambiguous
  (a chunk could have held >8 of the row's top-17, or duplicate indices).
"""

import numpy as np

import concourse.bacc as bacc
import concourse.mybir as mybir
import concourse.tile as tile
from concourse.bass_utils import run_bass_kernel_spmd

N = 16384
D = 128
K = 16
NCORES = 8
RPC = N // NCORES  # 2048 rows per core
RT = 128  # rows per row-tile (partition dim)
NRT = RPC // RT  # 16 row tiles per core
CH = 512  # columns per matmul (one PSUM bank of fp32)
NCH = N // CH  # 32 matmul chunks
SCH = 2048  # columns per selection chunk (four banks -> one SBUF tile)
NSCH = N // SCH  # 8 selection chunks
CAND = NSCH * 8  # 64 candidates per row
NSEL = 24  # 3 rounds x 8 merged winners (need 17)
NEG = -3.0e38

_CACHE = {}


def _build(repeat=1, mode="full"):
    f32 = mybir.dt.float32
    u16 = mybir.dt.uint16

    nc = bacc.Bacc(
        "TRN2", target_bir_lowering=False, debug=False, num_devices=NCORES
    )

    xt = nc.dram_tensor("xt", [D, N], f32, kind="ExternalInput")
    xrt = nc.dram_tensor("xrt", [D, RPC], f32, kind="ExternalInput")
    pos_out = nc.dram_tensor("pos_out", [RPC, NSEL], u16, kind="ExternalOutput")
    val_out = nc.dram_tensor("val_out", [RPC, NSEL], f32, kind="ExternalOutput")
    cidx_out = nc.dram_tensor("cidx_out", [RPC, CAND], u16, kind="ExternalOutput")
    flag_out = nc.dram_tensor("flag_out", [RPC, 1], f32, kind="ExternalOutput")

    with tile.TileContext(nc) as tc:
        with (
            tc.tile_pool(name="persist", bufs=1) as persist,
            tc.tile_pool(name="work", bufs=4) as wpool,
            tc.tile_pool(name="cand", bufs=2) as cpool,
            tc.tile_pool(name="small", bufs=2) as spool,
        ):
            xt_sb = persist.tile([D, N], f32)
            nc.sync.dma_start(xt_sb[:, :], xt[:, :])
            xrt_sb = persist.tile([D, RPC], f32)
            nc.sync.dma_start(xrt_sb[:, :], xrt[:, :])

            ones_col = persist.tile([D, 1], f32)
            nc.vector.memset(ones_col[:, :], 1.0)
            ones_row = persist.tile([1, D], f32)
            nc.vector.memset(ones_row[:, :], 1.0)
            negsq = persist.tile([1, N], f32)

            # Prologue: negsq[n] = -sum_d xt[d, n]^2 via ones_col.T @ (xt*xt)
            with tc.tile_pool(name="psumq", bufs=2, space="PSUM") as pqpool:
                for c in range(NCH):
                    sl = slice(c * CH, (c + 1) * CH)
                    xsq = wpool.tile([D, CH], f32, tag="xsq")
                    nc.vector.tensor_mul(xsq[:, :], xt_sb[:, sl], xt_sb[:, sl])
                    pq = pqpool.tile([1, CH], f32, tag="pq")
                    nc.tensor.matmul(
                        pq[:, :], ones_col[:, :], xsq[:, :], start=True, stop=True
                    )
                    nc.scalar.activation(
                        negsq[:, sl],
                        pq[:, :],
                        mybir.ActivationFunctionType.Copy,
                        scale=-1.0,
                    )

            pspool = tc.alloc_tile_pool(name="psum", bufs=2, space="PSUM")
            HALVES = SCH // CH
            GRP = 8
            for t in [t for _ in range(repeat) for t in range(NRT)]:
                lhs = xrt_sb[:, t * RT : (t + 1) * RT]
                cand_v = cpool.tile([RT, CAND], f32, tag="cv")
                cand_i = cpool.tile([RT, CAND], u16, tag="ci")
                if mode in ("mm", "mm_max"):
                    nc.vector.memset(cand_i[:, :], 0)
                if mode == "mm":
                    nc.vector.memset(cand_v[:, :], 0.0)
                for g in range(NSCH // 2):
                    pss = []
                    for j in range(2):
                        w = g * 2 + j  # wide selection chunk
                        ps = pspool.tile([RT, SCH], f32, tag="ps")
                        for h in range(HALVES):
                            c = HALVES * w + h
                            sl = slice(c * CH, (c + 1) * CH)
                            nc.tensor.matmul(
                                ps[:, h * CH : (h + 1) * CH],
                                lhs,
                                xt_sb[:, sl],
                                start=True,
                                stop=(mode == "nok1"),
                            )
                        pss.append((w, ps))
                    if mode != "nok1":
                        for w, ps in pss:
                            for h in range(HALVES):
                                c = HALVES * w + h
                                sl = slice(c * CH, (c + 1) * CH)
                                nc.tensor.matmul(
                                    ps[:, h * CH : (h + 1) * CH],
                                    ones_row[:, :],
                                    negsq[:, sl],
                                    start=False,
                                    stop=True,
                                )
                    for w, ps in pss:
                        so = slice(w * 8, (w + 1) * 8)
                        if mode == "noevac":
                            nc.vector.max(cand_v[:, so], ps[:, :])
                            nc.vector.max_index(
                                cand_i[:, so], cand_v[:, so], ps[:, :]
                            )
                            continue
                        if mode == "mm":
                            continue
                        pc = wpool.tile([RT, SCH], f32, tag="pc")
                        nc.scalar.activation(
                            pc[:, :], ps[:, :], mybir.ActivationFunctionType.Copy
                        )
                        nc.vector.max(cand_v[:, so], pc[:, :])
                        if mode not in ("mm_max",):
                            nc.vector.max_index(cand_i[:, so], cand_v[:, so], pc[:, :])

                # Merge: top-24 of the 256 candidate values (17 needed).
                v24 = spool.tile([RT, NSEL], f32, tag="v24")
                p24 = spool.tile([RT, NSEL], u16, tag="p24")
                mwork = cpool.tile([RT, CAND], f32, tag="mwork")
                for r in range(3):
                    so = slice(r * 8, (r + 1) * 8)
                    src = cand_v if r == 0 else mwork
                    nc.vector.max(v24[:, so], src[:, :])
                    nc.vector.max_index(p24[:, so], v24[:, so], src[:, :])
                    if r < 2:
                        nc.vector.match_replace(
                            mwork[:, :], v24[:, so], src[:, :], NEG
                        )

                # Ambiguity flag: some chunk's 8th-best still beats our 17th
                # winner => that chunk may have hidden a real neighbor.
                ch8 = cand_v.rearrange("p (c e) -> p c e", e=8)[:, :, 7]
                m8 = spool.tile([RT, 8], f32, tag="m8")
                nc.vector.max(m8[:, :], ch8)
                fl = spool.tile([RT, 1], f32, tag="fl")
                nc.vector.tensor_scalar(
                    fl[:, :],
                    m8[:, 0:1],
                    v24[:, K : K + 1],
                    None,
                    op0=mybir.AluOpType.is_ge,
                )

                rs = slice(t * RT, (t + 1) * RT)
                nc.sync.dma_start(pos_out[rs, :], p24[:, :])
                nc.sync.dma_start(val_out[rs, :], v24[:, :])
                nc.sync.dma_start(cidx_out[rs, :], cand_i[:, :])
                nc.sync.dma_start(flag_out[rs, :], fl[:, :])
            pspool.release()

    nc.compile()
    return nc


def _exact_rows(x, rows):
    """Recompute flagged rows with the reference's own f32 pipeline on CPU.

    Uses 2048-row-block jax GEMMs, which reproduce the full-matrix XLA:CPU
    GEMM rounding bitwise, so near-tie orderings match the reference exactly
    (verified empirically). Stable argsort == top_k tie-breaking."""
    import jax
    import jax.numpy as jnp

    out = np.empty((len(rows), K), np.int32)
    B = 2048
    with jax.default_device(jax.devices("cpu")[0]):
        xj = jnp.asarray(x)
        sq = jnp.sum(xj * xj, axis=1)
        for blk in sorted({int(r) // B for r in rows}):
            sl = slice(blk * B, (blk + 1) * B)
            d2 = sq[sl][:, None] + sq[None, :] - 2.0 * (xj[sl] @ xj.T)
            dist = np.asarray(jnp.sqrt(jnp.maximum(d2, 0.0)))
            for j, r in enumerate(rows):
                if blk * B <= r < (blk + 1) * B:
                    o = np.argsort(dist[r - blk * B], kind="stable")
                    out[j] = o[1 : K + 1].astype(np.int32)
    return out


def kernel(x, k):
    x = np.ascontiguousarray(np.asarray(x, dtype=np.float32))
    assert x.shape == (N, D)
    assert int(k) == K

    if "nc" not in _CACHE:
        _CACHE["nc"] = _build()
    nc = _CACHE["nc"]

    xt_full = np.ascontiguousarray(x.T)
    in_maps = []
    for i in range(NCORES):
        in_maps.append(
            {
                "xt": xt_full,
                # lhsT carries 2*x so the matmul yields 2 x_m.x_n (exact in fp32)
                "xrt": np.ascontiguousarray((2.0 * x[i * RPC : (i + 1) * RPC, :]).T),
            }
        )

    res = run_bass_kernel_spmd(nc, in_maps, core_ids=list(range(NCORES)))

    out = np.empty((N, K), np.int32)
    bad_rows = []
    for i, r in enumerate(res.results):
        pos = r["pos_out"].astype(np.int64)  # [RPC, NSEL]
        vals = r["val_out"]  # [RPC, NSEL] f32, descending
        cidx = r["cidx_out"].astype(np.int64)  # [RPC, CAND]
        flags = r["flag_out"].reshape(RPC)
        sel = pos[:, 1 : K + 1]  # slot 0 = self
        selc = np.clip(sel, 0, CAND - 1)
        loc = np.take_along_axis(cidx, selc, axis=1)
        glob = loc + (selc >> 3) * SCH
        base = i * RPC
        out[base : base + RPC] = glob.astype(np.int32)

        rows_self = np.arange(base, base + RPC)[:, None]
        dup = (np.sort(glob, axis=1)[:, 1:] == np.sort(glob, axis=1)[:, :-1]).any(1)
        # near-tie: any adjacent gap among neighbor slots 1..17 below eps
        gaps = vals[:, 1 : K + 1] - vals[:, 2 : K + 2]
        tie = (gaps < 1e-3).any(1)
        bad = (
            (flags != 0)
            | dup
            | tie
            | (sel >= CAND).any(1)
            | (glob >= N).any(1)
            | (glob == rows_self).any(1)
        )
        bad_rows.extend((base + np.nonzero(bad)[0]).tolist())

    if bad_rows:
        out[bad_rows] = _exact_rows(x, np.asarray(bad_rows))

    return out


# revision 24
# speedup vs baseline: 1.0870x; 1.0870x over previous
"""KNN graph kernel for Trainium2 (8 NeuronCores).

Problem: x [16384, 128] f32 -> indices of the 16 nearest neighbors per row
(excluding self), int32 [16384, 16], matching
    d2 = |x_i|^2 + |x_j|^2 - 2 x_i.x_j ;  top_k(-sqrt(max(d2,0)), 17)[:, 1:17]

Approach (per core, rows sharded 2048/core):
  score p[m, n] = 2 x_m.x_n - |x_n|^2  (row-monotone transform of -d2, so
  per-row ordering matches; self is the strict row max by ~|x_m|^2 margin).
  PE: fp32 matmuls produce p in PSUM [128, 512] chunks (K=128 main matmul
  plus a K=1 accumulate folding in -|x_n|^2).
  DVE: per chunk, Max (top-8 values) + MaxIndex (their positions); then a
  3-round Max/MaxIndex/MatchReplace merge over the 32*8=256 candidates
  gives the top-17 (slot 0 = self).
  Host: maps candidate positions back to global column indices, drops the
  self slot, and exactly recomputes the rare rows flagged as ambiguous
  (a chunk could have held >8 of the row's top-17, or duplicate indices).
"""

import numpy as np

import concourse.bacc as bacc
import concourse.mybir as mybir
import concourse.tile as tile
from concourse.bass_utils import run_bass_kernel_spmd

N = 16384
D = 128
K = 16
NCORES = 8
RPC = N // NCORES  # 2048 rows per core
RT = 128  # rows per row-tile (partition dim)
NRT = RPC // RT  # 16 row tiles per core
CH = 512  # columns per matmul (one PSUM bank of fp32)
NCH = N // CH  # 32 matmul chunks
SCH = 2048  # columns per selection chunk (four banks -> one SBUF tile)
NSCH = N // SCH  # 8 selection chunks
CAND = NSCH * 8  # 64 candidates per row
NSEL = 24  # 3 rounds x 8 merged winners (need 17)
NEG = -3.0e38

_CACHE = {}


def _build(repeat=1, mode="full"):
    f32 = mybir.dt.float32
    u16 = mybir.dt.uint16

    nc = bacc.Bacc(
        "TRN2", target_bir_lowering=False, debug=False, num_devices=NCORES
    )

    xt = nc.dram_tensor("xt", [D, N], f32, kind="ExternalInput")
    xrt = nc.dram_tensor("xrt", [D, RPC], f32, kind="ExternalInput")
    pos_out = nc.dram_tensor("pos_out", [RPC, NSEL], u16, kind="ExternalOutput")
    val_out = nc.dram_tensor("val_out", [RPC, NSEL], f32, kind="ExternalOutput")
    cidx_out = nc.dram_tensor("cidx_out", [RPC, CAND], u16, kind="ExternalOutput")
    flag_out = nc.dram_tensor("flag_out", [RPC, 1], f32, kind="ExternalOutput")

    with tile.TileContext(nc) as tc:
        with (
            tc.tile_pool(name="persist", bufs=1) as persist,
            tc.tile_pool(name="work", bufs=4) as wpool,
            tc.tile_pool(name="cand", bufs=2) as cpool,
            tc.tile_pool(name="small", bufs=2) as spool,
        ):
            xt_sb = persist.tile([D, N], f32)
            nc.sync.dma_start(xt_sb[:, :], xt[:, :])
            xrt_sb = persist.tile([D, RPC], f32)
            nc.sync.dma_start(xrt_sb[:, :], xrt[:, :])

            ones_col = persist.tile([D, 1], f32)
            nc.vector.memset(ones_col[:, :], 1.0)
            ones_row = persist.tile([1, D], f32)
            nc.vector.memset(ones_row[:, :], 1.0)
            negsq = persist.tile([1, N], f32)

            # Prologue: negsq[n] = -sum_d xt[d, n]^2 via ones_col.T @ (xt*xt)
            with tc.tile_pool(name="psumq", bufs=2, space="PSUM") as pqpool:
                for c in range(NCH):
                    sl = slice(c * CH, (c + 1) * CH)
                    xsq = wpool.tile([D, CH], f32, tag="xsq")
                    nc.vector.tensor_mul(xsq[:, :], xt_sb[:, sl], xt_sb[:, sl])
                    pq = pqpool.tile([1, CH], f32, tag="pq")
                    nc.tensor.matmul(
                        pq[:, :], ones_col[:, :], xsq[:, :], start=True, stop=True
                    )
                    nc.scalar.activation(
                        negsq[:, sl],
                        pq[:, :],
                        mybir.ActivationFunctionType.Copy,
                        scale=-1.0,
                    )

            pspool = tc.alloc_tile_pool(name="psum", bufs=2, space="PSUM")
            HALVES = SCH // CH
            GRP = 8
            for t in [t for _ in range(repeat) for t in range(NRT)]:
                lhs = xrt_sb[:, t * RT : (t + 1) * RT]
                cand_v = cpool.tile([RT, CAND], f32, tag="cv")
                cand_i = cpool.tile([RT, CAND], u16, tag="ci")
                if mode in ("mm", "mm_max"):
                    nc.vector.memset(cand_i[:, :], 0)
                if mode == "mm":
                    nc.vector.memset(cand_v[:, :], 0.0)
                for g in range(NSCH // 2):
                    pss = []
                    for j in range(2):
                        w = g * 2 + j  # wide selection chunk
                        ps = pspool.tile([RT, SCH], f32, tag="ps")
                        for h in range(HALVES):
                            c = HALVES * w + h
                            sl = slice(c * CH, (c + 1) * CH)
                            nc.tensor.matmul(
                                ps[:, h * CH : (h + 1) * CH],
                                lhs,
                                xt_sb[:, sl],
                                start=True,
                                stop=(mode == "nok1"),
                            )
                        pss.append((w, ps))
                    if mode != "nok1":
                        for w, ps in pss:
                            for h in range(HALVES):
                                c = HALVES * w + h
                                sl = slice(c * CH, (c + 1) * CH)
                                nc.tensor.matmul(
                                    ps[:, h * CH : (h + 1) * CH],
                                    ones_row[:, :],
                                    negsq[:, sl],
                                    start=False,
                                    stop=True,
                                )
                    for w, ps in pss:
                        so = slice(w * 8, (w + 1) * 8)
                        if mode == "noevac":
                            nc.vector.max(cand_v[:, so], ps[:, :])
                            nc.vector.max_index(
                                cand_i[:, so], cand_v[:, so], ps[:, :]
                            )
                            continue
                        if mode == "mm":
                            continue
                        pc = wpool.tile([RT, SCH], f32, tag="pc")
                        nc.scalar.activation(
                            pc[:, :], ps[:, :], mybir.ActivationFunctionType.Copy
                        )
                        nc.vector.max(cand_v[:, so], pc[:, :])
                        if mode not in ("mm_max",):
                            nc.vector.max_index(cand_i[:, so], cand_v[:, so], pc[:, :])

                # Merge: top-24 of the 256 candidate values (17 needed).
                v24 = spool.tile([RT, NSEL], f32, tag="v24")
                p24 = spool.tile([RT, NSEL], u16, tag="p24")
                mwork = cpool.tile([RT, CAND], f32, tag="mwork")
                for r in range(3):
                    so = slice(r * 8, (r + 1) * 8)
                    src = cand_v if r == 0 else mwork
                    nc.vector.max(v24[:, so], src[:, :])
                    nc.vector.max_index(p24[:, so], v24[:, so], src[:, :])
                    if r < 2:
                        nc.vector.match_replace(
                            mwork[:, :], v24[:, so], src[:, :], NEG
                        )

                # Ambiguity flag: some chunk's 8th-best still beats our 17th
                # winner => that chunk may have hidden a real neighbor.
                ch8 = cand_v.rearrange("p (c e) -> p c e", e=8)[:, :, 7]
                m8 = spool.tile([RT, 8], f32, tag="m8")
                nc.vector.max(m8[:, :], ch8)
                fl = spool.tile([RT, 1], f32, tag="fl")
                nc.vector.tensor_scalar(
                    fl[:, :],
                    m8[:, 0:1],
                    v24[:, K : K + 1],
                    None,
                    op0=mybir.AluOpType.is_ge,
                )

                rs = slice(t * RT, (t + 1) * RT)
                nc.sync.dma_start(pos_out[rs, :], p24[:, :])
                nc.sync.dma_start(val_out[rs, :], v24[:, :])
                nc.sync.dma_start(cidx_out[rs, :], cand_i[:, :])
                nc.sync.dma_start(flag_out[rs, :], fl[:, :])
            pspool.release()

    nc.compile()
    return nc


def _exact_rows(x, rows):
    """Recompute flagged rows with the reference's own f32 pipeline on CPU.

    Uses 2048-row-block jax GEMMs, which reproduce the full-matrix XLA:CPU
    GEMM rounding bitwise, so near-tie orderings match the reference exactly
    (verified empirically). Stable argsort == top_k tie-breaking."""
    import jax
    import jax.numpy as jnp

    out = np.empty((len(rows), K), np.int32)
    B = 2048
    with jax.default_device(jax.devices("cpu")[0]):
        xj = jnp.asarray(x)
        sq = jnp.sum(xj * xj, axis=1)
        for blk in sorted({int(r) // B for r in rows}):
            sl = slice(blk * B, (blk + 1) * B)
            d2 = sq[sl][:, None] + sq[None, :] - 2.0 * (xj[sl] @ xj.T)
            dist = np.asarray(jnp.sqrt(jnp.maximum(d2, 0.0)))
            for j, r in enumerate(rows):
                if blk * B <= r < (blk + 1) * B:
                    o = np.argsort(dist[r - blk * B], kind="stable")
                    out[j] = o[1 : K + 1].astype(np.int32)
    return out


def kernel(x, k):
    x = np.ascontiguousarray(np.asarray(x, dtype=np.float32))
    assert x.shape == (N, D)
    assert int(k) == K

    if "nc" not in _CACHE:
        _CACHE["nc"] = _build()
    nc = _CACHE["nc"]

    xt_full = np.ascontiguousarray(x.T)
    in_maps = []
    for i in range(NCORES):
        in_maps.append(
            {
                "xt": xt_full,
                # lhsT carries 2*x so the matmul yields 2 x_m.x_n (exact in fp32)
                "xrt": np.ascontiguousarray((2.0 * x[i * RPC : (i + 1) * RPC, :]).T),
            }
        )

    res = run_bass_kernel_spmd(nc, in_maps, core_ids=list(range(NCORES)))

    out = np.empty((N, K), np.int32)
    bad_rows = []
    for i, r in enumerate(res.results):
        pos = r["pos_out"].astype(np.int64)  # [RPC, NSEL]
        vals = r["val_out"]  # [RPC, NSEL] f32, descending
        cidx = r["cidx_out"].astype(np.int64)  # [RPC, CAND]
        flags = r["flag_out"].reshape(RPC)
        sel = pos[:, 1 : K + 1]  # slot 0 = self
        selc = np.clip(sel, 0, CAND - 1)
        loc = np.take_along_axis(cidx, selc, axis=1)
        glob = loc + (selc >> 3) * SCH
        base = i * RPC
        out[base : base + RPC] = glob.astype(np.int32)

        rows_self = np.arange(base, base + RPC)[:, None]
        dup = (np.sort(glob, axis=1)[:, 1:] == np.sort(glob, axis=1)[:, :-1]).any(1)
        # near-tie: any adjacent gap among neighbor slots 1..17 below eps
        gaps = vals[:, 1 : K + 1] - vals[:, 2 : K + 2]
        tie = (gaps < 1e-3).any(1)
        bad = (
            (flags != 0)
            | dup
            | tie
            | (sel >= CAND).any(1)
            | (glob >= N).any(1)
            | (glob == rows_self).any(1)
        )
        bad_rows.extend((base + np.nonzero(bad)[0]).tolist())

    if bad_rows:
        out[bad_rows] = _exact_rows(x, np.asarray(bad_rows))

    return out
